# revision 12
# baseline (speedup 1.0000x reference)
"""Trainium2 Bass kernel for nn_Attention_4183298146960.

GQA causal attention layer: B=2, S=2048, HIDDEN=2048, 16 q heads / 4 kv heads,
head_dim=128, RoPE (interleaved pairs), causal softmax, output projection.

Sharding (8 cores, SPMD-uniform program, batch-split tensor parallel):
  core c owns batch b = c//4 and q heads {4g..4g+3}, kv head g, where g = c%4.
  QKV + RoPE + attention fully local.  The output projection needs all 16
  heads' features, so per-core attention outputs are AllGathered within the
  4-core batch group ([[0,1,2,3],[4,5,6,7]]); each core then computes its
  512 output columns.

v2 optimizations over the first working version (331.8us):
  * startup: ft-major wqk layout + chunk-major x layout give 4-16KB DMA
    lines; preamble DMAs ordered by first-use; ~24 dummy matmuls on zeroed
    scratch absorb the initial DMA wait AND warm the HAM clock gate
    (PE otherwise runs its first ~25us at 1.2GHz).
  * attention inner loop: scores for the head PAIR land in one 2-bank PSUM
    tile -> single fused exp (halves ACT's 352-cycle/call overhead); the
    causal-diagonal tiles r>=2 are column-trimmed (packed) so score/exp/
    mask/PV all skip fully-masked columns; scores are emitted one k-tile
    ahead of PV so exp latency hides under PE work; denominator partial
    sums moved from DVE to GpSimd (DVE was 40% busy and on the exp->PV
    critical path via the mask multiply).
  * tail: the last chunk's AllGather is split per head-pair (first half
    triggers mid-attention); W_o for chunk 2 is only half-interleaved into
    att(3) so ~7us of real work remains to cover the final AG flight;
    last-chunk W_o runs as two 8-tile contraction passes combined via DVE.
  * rope-swap DMAs moved to the vector queue, attention stores to gpsimd,
    W_o output stores to vector, keeping the sync queue exclusively for
    bulk loads (x, weights, asb).

Layouts on device (partition dim first):
  feature-major qT/kT/vT [head_dim, tokens]; v is PE-transposed to
  token-major [tokens, head_dim] for PV.  qcat is chunk-major
  [128, qt(4), hp(2), hh(2), 512] so a head pair's chunk is contiguous.
  Scores are computed transposed [k, q] so softmax needs no max-subtraction
  and the denominator is a ones-matmul; probabilities stay unnormalized
  until after PV.  RoPE head dims are permuted [even | odd] via host-side
  W row permutation so the rotation is a 64-partition swap + DVE ops.
"""

from collections import deque

import numpy as np
import ml_dtypes

import concourse.bass as bass
import concourse.mybir as mybir
import concourse.tile as tile
from concourse import bacc
from concourse.bass_utils import run_bass_kernel_spmd

BF16 = ml_dtypes.bfloat16

HEADS = 16
KV_HEADS = 4
HIDDEN = 2048
HD = 128
S = 2048
B = 2
HT = HIDDEN // 128             # 16 hidden tiles
NH = 4                         # local q heads per core
SCALE = 1.0 / float(np.sqrt(HD))
RG = [[0, 1, 2, 3], [4, 5, 6, 7]]

# mask block offsets inside msk (all pair-layout, see _prep_inputs)
MSK_OFF = {0: 0, 1: 1024, 2: 2048, 3: 2560}
MSK_COLS = 2816

_COMPILED = None


def _build():
    dt = mybir.dt
    nc = bacc.Bacc("TRN2", target_bir_lowering=False, debug=False, num_devices=8)

    xT = nc.dram_tensor("xT", [128, 4, HT, 512], dt.bfloat16, kind="ExternalInput")
    wqk = nc.dram_tensor("wqk", [128, 6, HT, 128], dt.bfloat16, kind="ExternalInput")
    wo = nc.dram_tensor("wo", [128, HT, 512], dt.bfloat16, kind="ExternalInput")
    cc = nc.dram_tensor("cc", [128, S], dt.bfloat16, kind="ExternalInput")
    ss = nc.dram_tensor("ss", [128, S], dt.bfloat16, kind="ExternalInput")
    msk = nc.dram_tensor("msk", [128, MSK_COLS], dt.bfloat16, kind="ExternalInput")
    ones128 = nc.dram_tensor("ones128", [128, 128], dt.bfloat16, kind="ExternalInput")
    ident = nc.dram_tensor("ident", [128, 128], dt.bfloat16, kind="ExternalInput")
    outT = nc.dram_tensor("outT", [512, S], dt.float32, kind="ExternalOutput")

    mult = mybir.AluOpType.mult
    add = mybir.AluOpType.add
    Exp = mybir.ActivationFunctionType.Exp

    with tile.TileContext(nc) as tc:
        with (
            tc.tile_pool(name="const", bufs=1) as constp,
            tc.tile_pool(name="dram", bufs=1, space="DRAM") as dram,
            tc.tile_pool(name="xp", bufs=2) as xp,
            tc.tile_pool(name="rp", bufs=2) as rp,
            tc.tile_pool(name="probs", bufs=3) as probs,
            tc.tile_pool(name="smallp", bufs=4) as smallp,
            tc.tile_pool(name="ap", bufs=2) as apool,
            tc.tile_pool(name="wosb", bufs=2) as wosb,
            tc.tile_pool(name="outp", bufs=2) as outp,
            tc.tile_pool(name="qkps", bufs=2, space="PSUM") as qkps,
            tc.tile_pool(name="spool", bufs=2, space="PSUM") as spool,
            tc.tile_pool(name="pvp", bufs=1, space="PSUM") as pvp,
        ):
            qcat = constp.tile([128, NH * S], dt.bfloat16)  # [qt][hp][hh][512]
            kT = constp.tile([128, S], dt.bfloat16)
            vsb = constp.tile([128, S], dt.bfloat16)        # token-major v tiles
            wo_sb = constp.tile([128, HT, 512], dt.bfloat16)
            msk_sb = constp.tile([128, MSK_COLS], dt.bfloat16)
            ones_sb = constp.tile([128, 128], dt.bfloat16)
            ident_sb = constp.tile([128, 128], dt.bfloat16)
            wqk_sb = constp.tile([128, 6, HT, 128], dt.bfloat16)
            cc_sb = constp.tile([128, S], dt.bfloat16)
            ss_sb = constp.tile([128, S], dt.bfloat16)
            scratch = constp.tile([128, 512], dt.bfloat16)

            # ---- HAM warm-up + DMA-wait absorber: dummy matmuls on zeros.
            nc.vector.memset(scratch[:], 0.0)
            for _ in range(24):
                wps = qkps.tile([128, 512], dt.float32, name="wps", tag="qk")
                nc.tensor.matmul(
                    wps[:], lhsT=scratch[:, 0:128], rhs=scratch[:],
                    start=True, stop=True,
                )

            # ---- preamble DMAs, ordered by first use (sync queue = bulk loads)
            x_tiles = {}

            def load_x(tt):
                t = xp.tile([128, HT, 512], dt.bfloat16, name=f"x{tt}", tag="x")
                for q in range(4):
                    nc.sync.dma_start(
                        t[:, q * 4:(q + 1) * 4, :], xT[:, tt, q * 4:(q + 1) * 4, :]
                    )
                x_tiles[tt] = t

            nc.sync.dma_start(wqk_sb[:, 0, :, :], wqk[:, 0, :, :])
            load_x(0)
            nc.sync.dma_start(cc_sb[:], cc[:])
            nc.sync.dma_start(ss_sb[:], ss[:])
            for ft in range(1, 6):
                nc.sync.dma_start(wqk_sb[:, ft, :, :], wqk[:, ft, :, :])
            nc.sync.dma_start(ident_sb[:], ident[:])
            nc.sync.dma_start(msk_sb[:], msk[:])
            nc.sync.dma_start(ones_sb[:], ones128[:])
            load_x(1)
            nc.sync.dma_start(wo_sb[:], wo[:])

            class WoFiller:
                """Doles out W_o matmuls one at a time into attention gaps."""

                def __init__(self):
                    self.gens = deque()

                def _gen(self, qt, ag_out):
                    asb = wosb.tile([128, HT, 512], dt.bfloat16, name="asb", tag="asb", bufs=1)
                    for dtt in range(HT):
                        nc.sync.dma_start(asb[:, dtt, :], ag_out[dtt, :, :])
                    for _ in range(12):
                        yield            # let the AG + first asb DMAs land
                    for ct in range(4):
                        ps_o = qkps.tile([128, 512], dt.float32, name="pso", tag="qk")
                        for dtt in range(HT):
                            nc.tensor.matmul(
                                ps_o[:],
                                lhsT=wo_sb[:, dtt, ct * 128:(ct + 1) * 128],
                                rhs=asb[:, dtt, :],
                                start=(dtt == 0), stop=(dtt == HT - 1),
                            )
                            yield
                        o_sb = outp.tile([128, 512], dt.float32)
                        nc.vector.tensor_copy(o_sb[:], ps_o[:])
                        nc.sync.dma_start(
                            outT[ct * 128:(ct + 1) * 128, qt * 512:(qt + 1) * 512],
                            o_sb[:],
                        )

                def add(self, qt, ag_out):
                    self.gens.append(self._gen(qt, ag_out))

                def step(self, n):
                    while n > 0 and self.gens:
                        try:
                            next(self.gens[0])
                            n -= 1
                        except StopIteration:
                            self.gens.popleft()

                def flush(self):
                    while self.gens:
                        self.step(64)

            woq = WoFiller()

            def emit_scores(qt, hp, kt):
                """Score pair MMs + fused exp (+ causal mask).  Returns prob."""
                r = kt - 4 * qt
                qbase = qt * 2048 + hp * 1024
                sp = spool.tile([128, 1024], dt.float32, name="sp", tag="sp")
                if r >= 2:
                    w = 512 - 128 * r
                    trim = 128 * r
                else:
                    w = 512
                    trim = 0
                for hh in range(2):
                    spoff = hh * w if r >= 2 else hh * 512
                    qoff = qbase + hh * 512 + trim
                    nc.tensor.matmul(
                        sp[:, spoff:spoff + w],
                        lhsT=kT[:, kt * 128:(kt + 1) * 128],
                        rhs=qcat[:, qoff:qoff + w],
                        start=True, stop=True,
                    )
                n = 2 * w
                prob = probs.tile([128, 1024], dt.bfloat16, name="prob")
                if r >= 0:
                    stg = probs.tile([128, 1024], dt.bfloat16, name="stg", tag="stg", bufs=2)
                    nc.scalar.activation(stg[:, 0:n], sp[:, 0:n], Exp, scale=SCALE)
                    mo = MSK_OFF[r]
                    nc.vector.tensor_tensor(
                        prob[:, 0:n], stg[:, 0:n], msk_sb[:, mo:mo + n], mult
                    )
                else:
                    nc.scalar.activation(prob[:, 0:n], sp[:, 0:n], Exp, scale=SCALE)
                return prob

            def emit_att(qt):
                """Attention for 512-token chunk qt (k/v tiles <= 4qt+3 ready)."""
                kts = 4 * qt + 4
                last = qt == 3
                stepn = 3 if last else 5
                if last:
                    chunks = [dram.tile([256, 512], dt.bfloat16, name=f"at3{h}")
                              for h in range(2)]
                    ag_outs = [dram.tile([8, 128, 512], dt.bfloat16,
                                         name=f"ag3{h}")
                               for h in range(2)]
                else:
                    chunk = dram.tile([512, 512], dt.bfloat16, name=f"attnc{qt}")
                    ag_out = dram.tile([HT, 128, 512], dt.bfloat16,
                                       name=f"agout{qt}")
                for hp in range(2):
                    pv = pvp.tile([128, 1024], dt.float32, name="pv")
                    acc = smallp.tile([128, 1024], dt.bfloat16, tag="acc", bufs=2,
                                      name="acc")
                    accB = None
                    if qt >= 2:
                        # second denominator chain on gpsimd unloads DVE
                        accB = smallp.tile([128, 1024], dt.bfloat16, tag="accb",
                                           bufs=2, name="accB")
                        nc.vector.memset(accB[:], 0.0)
                    prob_next = emit_scores(qt, hp, 0)
                    for kt in range(kts):
                        r = kt - 4 * qt
                        prob = prob_next
                        if kt + 1 < kts:
                            prob_next = emit_scores(qt, hp, kt + 1)
                        woq.step(stepn)
                        if r >= 2:
                            w = 512 - 128 * r
                            trim = 128 * r
                        else:
                            w = 512
                            trim = 0
                        for hh in range(2):
                            poff = hh * w if r >= 2 else hh * 512
                            nc.tensor.matmul(
                                pv[:, hh * 512 + trim: (hh + 1) * 512],
                                lhsT=vsb[:, kt * 128:(kt + 1) * 128],
                                rhs=prob[:, poff:poff + w],
                                start=(kt == 0), stop=(kt == kts - 1),
                            )
                        if kt == 0:
                            nc.vector.tensor_copy(acc[:], prob[:])
                        elif r < 2:
                            if accB is not None and kt % 2 == 1:
                                nc.gpsimd.tensor_add(accB[:], accB[:], prob[:])
                            else:
                                nc.vector.tensor_add(acc[:], acc[:], prob[:])
                        else:
                            for hh in range(2):
                                a0 = hh * 512 + trim
                                nc.vector.tensor_add(
                                    acc[:, a0:hh * 512 + 512],
                                    acc[:, a0:hh * 512 + 512],
                                    prob[:, hh * w:(hh + 1) * w],
                                )
                    for hh in range(2):
                        # partition-reduce + broadcast denominators in one matmul
                        den_ps = spool.tile([128, 512], dt.float32, name="den", tag="sp")
                        nc.tensor.matmul(
                            den_ps[:], lhsT=ones_sb[:],
                            rhs=acc[:, hh * 512:(hh + 1) * 512],
                            start=True, stop=(accB is None),
                        )
                        if accB is not None:
                            nc.tensor.matmul(
                                den_ps[:], lhsT=ones_sb[:],
                                rhs=accB[:, hh * 512:(hh + 1) * 512],
                                start=False, stop=True,
                            )
                        den_sb = smallp.tile([128, 512], dt.float32, tag="den",
                                             bufs=4, name="den_sb")
                        nc.vector.reciprocal_approx_fast(den_sb[:], den_ps[:])
                        attn_sb = apool.tile([128, 512], dt.bfloat16)
                        nc.vector.tensor_tensor(
                            attn_sb[:], pv[:, hh * 512:(hh + 1) * 512], den_sb[:],
                            mult,
                        )
                        woq.step(stepn)
                        if last:
                            nc.sync.dma_start(
                                chunks[hp][hh * 128:(hh + 1) * 128, :], attn_sb[:]
                            )
                        else:
                            lh = 2 * hp + hh
                            nc.sync.dma_start(
                                chunk[lh * 128:(lh + 1) * 128, :], attn_sb[:]
                            )
                    if last:
                        # half-AG fires as soon as this head pair is done
                        nc.gpsimd.collective_compute(
                            "AllGather", mybir.AluOpType.bypass, replica_groups=RG,
                            ins=[chunks[hp].opt()], outs=[ag_outs[hp].opt()],
                        )
                if last:
                    return ag_outs
                nc.gpsimd.collective_compute(
                    "AllGather", mybir.AluOpType.bypass, replica_groups=RG,
                    ins=[chunk.opt()], outs=[ag_out.opt()],
                )
                return ag_out

            pending_wo = []
            for tt in range(4):
                if 1 <= tt < 3:
                    load_x(tt + 1)
                x_sb = x_tiles[tt]
                tsl = bass.ts(tt, 512)
                sbv = None
                for ft in (5, 4, 0, 1, 2, 3):  # v,k first; transpose deferred
                    ps = qkps.tile([128, 512], dt.float32, name="qk", tag="qk")
                    for ht in range(HT):
                        nc.tensor.matmul(
                            ps[:],
                            lhsT=wqk_sb[:, ft, ht, :],
                            rhs=x_sb[:, ht, :],
                            start=(ht == 0),
                            stop=(ht == HT - 1),
                        )
                    if ft == 5:
                        sbv = rp.tile([128, 512], dt.bfloat16, tag="sbv", name="sbv")
                        nc.vector.tensor_copy(sbv[:], ps[:])
                        continue
                    sbq = rp.tile([128, 512], dt.bfloat16)
                    nc.scalar.copy(sbq[:], ps[:])
                    tmp = rp.tile([128, 512], dt.bfloat16)
                    # scalar queue: keeps sync queue free for bulk loads
                    nc.scalar.dma_start(tmp[0:64, :], sbq[64:128, :])
                    nc.scalar.dma_start(tmp[64:128, :], sbq[0:64, :])
                    qcc = rp.tile([128, 512], dt.bfloat16)
                    nc.vector.tensor_tensor(qcc[:], sbq[:], cc_sb[:, tsl], mult)
                    qss = rp.tile([128, 512], dt.bfloat16)
                    nc.vector.tensor_tensor(qss[:], tmp[:], ss_sb[:, tsl], mult)
                    if ft < 4:
                        qb = tt * 2048 + (ft // 2) * 1024 + (ft % 2) * 512
                        dst = qcat[:, qb:qb + 512]
                    else:
                        dst = kT[:, tsl]
                    nc.vector.tensor_tensor(dst, qcc[:], qss[:], add)
                # v transpose deferred to block end: sbv copy had ~15us to land
                ps_t = qkps.tile([128, 1024], dt.bfloat16, name="qkt", tag="qk")
                for st in range(4):
                    nc.tensor.transpose(
                        ps_t[:, st * 128:(st + 1) * 128],
                        sbv[:, st * 128:(st + 1) * 128],
                        ident_sb[:],
                    )
                nc.vector.tensor_copy(vsb[:, tsl], ps_t[:, 0:512])
                # W_o for gathered chunks interleaves into this chunk's attn
                while len(pending_wo) >= 2:
                    woq.add(*pending_wo.pop(0))
                ags = emit_att(tt)
                if tt < 3:
                    pending_wo.append((tt, ags))

            ag3a, ag3b = ags

            # ---- tail: W_o(2) covers the AG3 flights, then the two
            # half-contraction passes for chunk 3.
            for item in pending_wo:
                woq.add(*item)
            pending_wo.clear()
            woq.flush()

            def wo3_pass(ag_half, dtts, o_parts, combine):
                asb = wosb.tile([128, 8, 512], dt.bfloat16, tag="asbh", name="asbh")
                for i in range(8):
                    nc.sync.dma_start(asb[:, i, :], ag_half[i, :, :])
                for ct in range(4):
                    ps_o = qkps.tile([128, 512], dt.float32, name="pso", tag="qk")
                    for i, dtt in enumerate(dtts):
                        nc.tensor.matmul(
                            ps_o[:],
                            lhsT=wo_sb[:, dtt, ct * 128:(ct + 1) * 128],
                            rhs=asb[:, i, :],
                            start=(i == 0), stop=(i == 7),
                        )
                    if not combine:
                        op = smallp.tile([128, 512], dt.float32, tag="den",
                                         bufs=4, name="opart")
                        nc.vector.tensor_copy(op[:], ps_o[:])
                        o_parts.append(op)
                    else:
                        o_sb = outp.tile([128, 512], dt.float32)
                        nc.vector.tensor_tensor(o_sb[:], ps_o[:], o_parts[ct][:], add)
                        nc.sync.dma_start(
                            outT[ct * 128:(ct + 1) * 128, 3 * 512:4 * 512], o_sb[:]
                        )

            o_parts = []
            wo3_pass(ag3a, [4 * r + l for r in range(4) for l in range(2)],
                     o_parts, combine=False)
            wo3_pass(ag3b, [4 * r + 2 + l for r in range(4) for l in range(2)],
                     o_parts, combine=True)
    nc.compile()
    return nc


# host-side input prep ------------------------------------------------------

_PERM = np.concatenate([np.arange(0, HD, 2), np.arange(1, HD, 2)])


def _rope_tables():
    freq = 1.0 / (10000.0 ** (np.arange(0, HD, 2, dtype=np.float64) / HD))
    pos = np.arange(S, dtype=np.float64)
    ang = np.outer(pos, freq)                       # [S, 64]
    cos = np.cos(ang).T.astype(np.float32)          # [64, S]
    sin = np.sin(ang).T.astype(np.float32)
    cc1 = np.concatenate([cos, cos], 0)             # [128, S]
    ss1 = np.concatenate([-sin, sin], 0)            # [128, S]
    return cc1.astype(BF16), ss1.astype(BF16)


def _masks():
    ii = np.arange(128)[:, None]
    jj = np.arange(512)[None, :]
    tri0 = (jj >= ii).astype(np.float32)            # [128, 512]
    tri1 = (jj >= ii + 128).astype(np.float32)
    p0 = np.concatenate([tri0, tri0], 1)            # r=0 pair   [128,1024]
    p1 = np.concatenate([tri1, tri1], 1)            # r=1 pair   [128,1024]
    p2 = np.concatenate([tri0[:, :256], tri0[:, :256]], 1)   # r=2 packed
    p3 = np.concatenate([tri0[:, :128], tri0[:, :128]], 1)   # r=3 packed
    return np.concatenate([p0, p1, p2, p3], 1).astype(BF16)  # [128, 2816]


def _prep_inputs(x, W_qkv, W_o):
    x = np.asarray(x, dtype=np.float32)
    W_qkv = np.asarray(W_qkv, dtype=np.float32)
    W_o = np.asarray(W_o, dtype=np.float32)

    xTb = []
    for b in range(B):
        # [128 hid-part, 4 chunk, 16 ht, 512 tok] -> 16KB contiguous lines
        xTb.append(np.ascontiguousarray(
            x[b].T.reshape(HT, 128, 4, 512).transpose(1, 2, 0, 3)
        ).astype(BF16))

    cc, ss = _rope_tables()
    mask = _masks()
    ones128 = np.ones((128, 128), dtype=np.float32).astype(BF16)
    ident = np.eye(128, dtype=np.float32).astype(BF16)

    in_maps = []
    for c in range(8):
        b, g = c // 4, c % 4
        qr = W_qkv[512 * g: 512 * (g + 1)]           # rows of q heads 4g..4g+3
        qr = qr.reshape(NH, HD, HIDDEN)[:, _PERM, :].reshape(512, HIDDEN)
        kr = W_qkv[HIDDEN + 128 * g: HIDDEN + 128 * (g + 1)][_PERM, :]
        vr = W_qkv[HIDDEN + 512 + 128 * g: HIDDEN + 512 + 128 * (g + 1)]
        wcat = np.concatenate([qr, kr, vr], 0)       # [768, 2048]
        # [128 hid-part, 6 ft, 16 ht, 128 feat] -> 4KB lines per ft
        wqkT = np.ascontiguousarray(
            wcat.reshape(6, 128, HT, 128).transpose(3, 0, 2, 1)
        ).astype(BF16)
        woT = np.ascontiguousarray(
            W_o[512 * g: 512 * (g + 1)].T.reshape(HT, 128, 512).transpose(1, 0, 2)
        ).astype(BF16)
        in_maps.append({
            "xT": xTb[b], "wqk": wqkT, "wo": woT,
            "cc": cc, "ss": ss, "msk": mask, "ones128": ones128, "ident": ident,
        })
    return in_maps


def kernel(x, W_qkv, W_o):
    global _COMPILED
    if _COMPILED is None:
        _COMPILED = _build()
    nc = _COMPILED
    in_maps = _prep_inputs(x, W_qkv, W_o)
    res = run_bass_kernel_spmd(nc, in_maps, list(range(8)))
    out = np.empty((B, S, HIDDEN), dtype=np.float32)
    for c in range(8):
        b, g = c // 4, c % 4
        oT = res.results[c]["outT"]                  # [512, 2048]
        out[b, :, 512 * g: 512 * (g + 1)] = oT.T
    return out
